# revision 91
# baseline (speedup 1.0000x reference)
"""Bamba attention decoder layer on 8 Trainium2 NeuronCores.

Sharding: tensor-parallel attention (4 q heads + 1 kv head per core),
AllToAll of attention context (delivers each core its token slice at a static
address), token-sliced o_proj + fused add/rmsnorm, AllGather of normed
activations, I-sharded SwiGLU MLP (1792 cols/core), ReduceScatter of
down-proj partials.

All matmul operands are bf16 (fp32 PSUM accumulation and fp32 rmsnorm
stats); the rmsnorm sum-of-squares accumulates on the otherwise-idle GpSimd
engine (one f32 ones-matmul per block for the partition reduction); the
softmax sum-of-exp accumulates on the PE via a ones-matmul instead of DVE
adds; causal masks are bf16 multiplies and diagonal score tiles are narrowed
to their causally live q-range. DMA issue order is engineered around the
FIFO sync queue: the first QKV matmul starts after ~0.6MB of traffic,
o_proj weights and attention-context slices stream in during attention (one
transposed-view DMA per head), the first gate/up weight pairs are emitted
late in the o_proj loop so their transfers fill the rmsnorm2 window, the
AllGather of normed activations is split in two k-halves with the first
gathered activation slices queued between them, gate/up/down weight streams
are ring-buffered with prefetch, and the first half of the SwiGLU
intermediate restages into SBUF while phase 4 still runs.
"""

import numpy as np
import ml_dtypes

import concourse.bacc as bacc
import concourse.mybir as mybir
import concourse.tile as tile
from concourse.bass_utils import run_bass_kernel_spmd
from concourse.masks import make_identity

NC = 8
S = 2048
H = 4096
HD = 128
NQ = 32
NKV = 8
I = 14336
QH = NQ // NC        # q heads per core = 4
IPC = I // NC        # intermediate cols per core = 1792
TPC = S // NC        # tokens per core = 256
EPS = 1e-5
THETA = 10000.0
SCALE = HD ** -0.5

F32 = mybir.dt.float32
BF16 = mybir.dt.bfloat16

KH = H // 128        # 32 k-tiles over H
NB = S // 512        # 4 token blocks of 512
MB_GU = IPC // 128   # 14 m tiles for gate (and for up)
KI = IPC // 128      # 14 k tiles over I per core

AF = mybir.ActivationFunctionType


def _phase1(nc, tc, g):
    """QKV matmul + rmsnorm1 stats + rope. Fills qT_sb/kT_sb/v_tok."""
    with (
        tc.tile_pool(name="p1sbuf", bufs=1) as p1s,
        tc.tile_pool(name="p1w", bufs=1) as p1w,
        tc.tile_pool(name="p1psum", bufs=1, space="PSUM") as p1p,
    ):
        wq_sb = p1w.tile([128, KH, (QH + 2) * 128], BF16, name="wq_sb")  # 6.3 MB
        nc.sync.dma_start(wq_sb[:, 0:2, :], g["wqkv"][:, 0:2, :])
        cos_sb = p1w.tile([128, S], BF16, name="cos_sb")
        sin_sb = p1w.tile([128, S], BF16, name="sin_sb")

        for nb in range(NB):
            ncols = slice(nb * 512, (nb + 1) * 512)
            st_ps = p1p.tile([1, 512], F32, name="st_ps", tag="st_ps")
            acc = p1s.tile([128, 512], F32, name="acc", tag="acc", bufs=1)
            nc.gpsimd.memset(acc[:], 0.0)
            mm_ps = []
            for m in range(QH + 2):
                t = p1p.tile([128, 512], F32, name=f"qkv_ps{m}", tag=f"qkv_ps{m}")
                mm_ps.append(t)
            for k in range(KH):
                if nb == 0:
                    # stagger the remaining weight chunks + rope tables behind
                    # the hb stream so the first matmuls start early
                    if k == 1:
                        nc.sync.dma_start(wq_sb[:, 2:8, :], g["wqkv"][:, 2:8, :])
                    elif k == 4:
                        nc.sync.dma_start(wq_sb[:, 8:16, :], g["wqkv"][:, 8:16, :])
                    elif k == 10:
                        nc.sync.dma_start(wq_sb[:, 16:24, :], g["wqkv"][:, 16:24, :])
                    elif k == 16:
                        nc.sync.dma_start(wq_sb[:, 24:32, :], g["wqkv"][:, 24:32, :])
                    elif k == 22:
                        nc.sync.dma_start(cos_sb[:], g["cosT"][:, :])
                    elif k == 26:
                        nc.sync.dma_start(sin_sb[:], g["sinT"][:, :])
                    elif k == 30:
                        nc.sync.dma_start(g["mask_sb"][:], g["masks"][:, :, :])
                hb = p1s.tile([128, 512], BF16, name="hb", tag="hb", bufs=4)
                nc.sync.dma_start(hb[:], g["hT"][k * 128:(k + 1) * 128, ncols])
                sq = p1s.tile([128, 512], BF16, name="sq", tag="sq", bufs=3)
                nc.vector.tensor_mul(sq[:], hb[:], hb[:])
                # accumulate the sum-of-squares on the (otherwise idle) Pool
                # engine; the cross-partition reduction happens once per block
                nc.gpsimd.tensor_add(acc[:], acc[:], sq[:])
                for m in range(QH + 2):
                    nc.tensor.matmul(
                        mm_ps[m][:], wq_sb[:, k, m * 128:(m + 1) * 128], hb[:],
                        start=(k == 0), stop=(k == KH - 1),
                    )
            # rmsnorm stats for this token block
            nc.tensor.matmul(st_ps[:], g["ones32"][:], acc[:], start=True, stop=True)
            std_row = p1s.tile([1, 512], F32, name="std_row", tag="std_row")
            nc.scalar.activation(std_row[:], st_ps[:], AF.Sqrt,
                                 bias=g["epsb"][:], scale=1.0 / H)
            rstd = p1s.tile([1, 512], F32, name="rstd", tag="rstd")
            nc.vector.reciprocal(rstd[:], std_row[:])
            rb32 = p1s.tile([128, 512], F32, name="rb32", tag="rb32")
            nc.gpsimd.partition_broadcast(rb32[:], rstd[:])
            rstdb = p1s.tile([1, 512], BF16, name="rstdb", tag="rstdb")
            nc.vector.tensor_copy(rstdb[:], rstd[:])
            rbb = p1s.tile([128, 512], BF16, name="rbb", tag="rbb")
            nc.gpsimd.partition_broadcast(rbb[:], rstdb[:])
            # 1/rms folded into the rope tables (per-token column scale)
            cos_s = p1s.tile([128, 512], BF16, name="cos_s", tag="cos_s")
            nc.vector.tensor_mul(cos_s[:], cos_sb[:, ncols], rbb[:])
            sin_s = p1s.tile([128, 512], BF16, name="sin_s", tag="sin_s")
            nc.vector.tensor_mul(sin_s[:], sin_sb[:, ncols], rbb[:])
            # evacuate the 5 rope-bound psums so the PE can start the next
            # token block while rope runs from SBUF
            qkc = p1s.tile([128, QH + 1, 512], BF16, name="qkc", tag="qkc", bufs=2)
            for m in range(QH + 1):
                nc.scalar.copy(qkc[:, m, :], mm_ps[m][:])
            for m in range(QH + 1):
                if m < QH:
                    d0 = g["qT_sb"][0:64, m, ncols]
                    d1 = g["qT_sb"][64:128, m, ncols]
                else:
                    d0 = g["kT_sb"][0:64, ncols]
                    d1 = g["kT_sb"][64:128, ncols]
                t0 = p1s.tile([64, 512], BF16, name="t0", tag="t0")
                nc.vector.tensor_mul(t0[:], qkc[0:64, m, :], cos_s[0:64, :])
                t1 = p1s.tile([64, 512], BF16, name="t1", tag="t1")
                nc.vector.tensor_mul(t1[:], qkc[64:128, m, :], sin_s[64:128, :])
                nc.vector.tensor_sub(d0, t0[:], t1[:])
                t2 = p1s.tile([64, 512], BF16, name="t2", tag="t0")
                nc.vector.tensor_mul(t2[:], qkc[64:128, m, :], cos_s[64:128, :])
                t3 = p1s.tile([64, 512], BF16, name="t3", tag="t1")
                nc.vector.tensor_mul(t3[:], qkc[0:64, m, :], sin_s[0:64, :])
                nc.vector.tensor_add(d1, t2[:], t3[:])
            vtmp = p1s.tile([128, 512], BF16, name="vtmp", tag="vtmp")
            nc.vector.tensor_mul(vtmp[:], mm_ps[QH + 1][:], rb32[:])
            tp = p1p.tile([128, 4, 128], BF16, name="tp", tag="tp")
            for j in range(4):
                nc.tensor.transpose(tp[:, j, :], vtmp[:, j * 128:(j + 1) * 128],
                                    g["ident"][:])
            nc.vector.tensor_copy(g["v_tok"][:, nb * 4:(nb + 1) * 4, :], tp[:])


def _phase2(nc, tc, g, with_collectives, rg):
    """Causal GQA attention. Sum-of-exp accumulated on PE via ones-matmul."""
    p2s = g["p2work"]
    with (
        tc.tile_pool(name="p2psum", bufs=1, space="PSUM") as p2p,
    ):
        for hh in range(QH):
            for qb in range(NB):
                qcols = slice(qb * 512, (qb + 1) * 512)
                nkt = 4 * qb + 4
                att_ps = p2p.tile([128, 512], F32, name="att_ps", tag="att_ps", bufs=2)
                sums_ps = p2p.tile([1, 512], F32, name="sums_ps", tag="sums_ps", bufs=2)
                for kt in range(nkt):
                    j = kt - 4 * qb
                    # diagonal tile j covers only q >= 128*j within the block
                    lo = 128 * j if j > 0 else 0
                    qs = slice(qb * 512 + lo, (qb + 1) * 512)
                    s_ps = p2p.tile([128, 512], F32, name="s_ps", tag="s_ps", bufs=4)
                    nc.tensor.matmul(
                        s_ps[:, lo:512], g["kT_sb"][:, kt * 128:(kt + 1) * 128],
                        g["qT_sb"][:, hh, qs], start=True, stop=True,
                    )
                    e = p2s.tile([128, 512], BF16, name="e", tag="e", bufs=8)
                    nc.scalar.activation(e[:, lo:512], s_ps[:, lo:512],
                                         AF.Exp, scale=SCALE)
                    if j >= 0:
                        nc.vector.tensor_mul(e[:, lo:512], e[:, lo:512],
                                             g["mask_sb"][:, j, lo:512])
                    nc.tensor.matmul(sums_ps[:, lo:512], g["ones"][:], e[:, lo:512],
                                     start=(kt == 0), stop=(kt == nkt - 1))
                    nc.tensor.matmul(att_ps[:, lo:512], g["v_tok"][:, kt, :],
                                     e[:, lo:512],
                                     start=(kt == 0), stop=(kt == nkt - 1))
                recip = p2s.tile([1, 512], F32, name="recip", tag="recip", bufs=2)
                nc.vector.reciprocal(recip[:], sums_ps[:])
                rb2 = p2s.tile([128, 512], F32, name="rb2", tag="rb2", bufs=2)
                nc.gpsimd.partition_broadcast(rb2[:], recip[:])
                anorm = p2s.tile([128, 512], BF16, name="anorm", tag="anorm", bufs=2)
                nc.vector.tensor_mul(anorm[:], att_ps[:], rb2[:])
                for half in range(2):
                    dst_core = qb * 2 + half
                    nc.sync.dma_start(
                        g[f"a2a_in{hh}"][dst_core, :, :],
                        anorm[:, half * 256:(half + 1) * 256],
                    )
            # ship this head's context while the next head computes
            if with_collectives:
                nc.gpsimd.collective_compute(
                    "AllToAll", mybir.AluOpType.bypass, replica_groups=rg,
                    ins=[g[f"a2a_in{hh}"].opt()], outs=[g[f"a2a_out{hh}"].opt()],
                )
            else:
                nc.sync.dma_start(g[f"a2a_out{hh}"][:, :, :], g[f"a2a_in{hh}"][:, :, :])
            # stage this head's o_proj input slices as they land (one DMA,
            # transposed view: [r, p, t] -> [p, r, t])
            nc.sync.dma_start(
                g["asl"][:, hh * 8:(hh + 1) * 8, :],
                g[f"a2a_out{hh}"][:, :, :].transpose([1, 0, 2]),
            )
            # opportunistic prefetch for phase 3
            if hh == 0:
                for kq in range(4):
                    nc.sync.dma_start(g["hsl"][:, kq * 8:(kq + 1) * 8, :],
                                      g["hT_slice"][:, kq * 8:(kq + 1) * 8, :])
            elif hh == 1:
                g["issue_wob"](0)
                g["issue_wob"](1)
            elif hh == 2:
                g["issue_wob"](2)
            elif hh == 3:
                g["issue_wob"](3)


def _phase3(nc, tc, g, with_collectives, rg):
    """Token-sliced o_proj + residual add + rmsnorm2 + AllGather of x2."""
    with (
        tc.tile_pool(name="p3sbuf", bufs=1) as p3s,
        tc.tile_pool(name="p3big", bufs=1) as p3b,
        tc.tile_pool(name="p3psum", bufs=1, space="PSUM") as p3p,
    ):
        res2 = p3b.tile([128, KH, TPC], BF16, name="res2")  # 2 MB
        st2_ps = p3p.tile([1, TPC], F32, name="st2_ps", tag="st2_ps")
        acc2 = p3s.tile([128, TPC], F32, name="acc2", tag="acc2", bufs=1)
        nc.gpsimd.memset(acc2[:], 0.0)
        # asl k index is head-major (hh*8+r): head 3's A2A lands last
        for m in range(KH):
            if m + 4 < KH:
                g["issue_wob"](m + 4)
            if m == 28 and "prefetch_gu01" in g:
                # first two gate/up weight pairs; emitted here (after all wob
                # issues) so their transfers fill the stats/x2 window and the
                # AllGather chain heads the queue at the phase boundary
                g["prefetch_gu01"]()
            wob = g["wob_tiles"][m]
            o_ps = p3p.tile([128, TPC], F32, name="o_ps", tag="o_ps", bufs=4)
            for k in range(KH):
                nc.tensor.matmul(o_ps[:], wob[:, k, :], g["asl"][:, k, :],
                                 start=(k == 0), stop=(k == KH - 1))
            hslm = p3s.tile([128, TPC], F32, name="hslm", tag="hslm", bufs=2)
            nc.vector.tensor_copy(hslm[:], g["hsl"][:, m, :])
            nc.vector.tensor_add(res2[:, m, :], o_ps[:], hslm[:])
            nc.sync.dma_start(g["res_out"][m * 128:(m + 1) * 128, :], res2[:, m, :])
            sq2 = p3s.tile([128, TPC], BF16, name="sq2", tag="sq2", bufs=2)
            nc.vector.tensor_mul(sq2[:], res2[:, m, :], res2[:, m, :])
            nc.gpsimd.tensor_add(acc2[:], acc2[:], sq2[:])
        nc.tensor.matmul(st2_ps[:], g["ones32"][:], acc2[:], start=True, stop=True)
        std2 = p3s.tile([1, TPC], F32, name="std2", tag="std2")
        nc.scalar.activation(std2[:], st2_ps[:], AF.Sqrt, bias=g["epsb"][:],
                             scale=1.0 / H)
        rstd2 = p3s.tile([1, TPC], F32, name="rstd2", tag="rstd2")
        nc.vector.reciprocal(rstd2[:], std2[:])
        rstd2b = p3s.tile([1, TPC], BF16, name="rstd2b", tag="rstd2b")
        nc.vector.tensor_copy(rstd2b[:], rstd2[:])
        rb3 = p3s.tile([128, TPC], BF16, name="rb3", tag="rb3")
        nc.gpsimd.partition_broadcast(rb3[:], rstd2b[:])
        x2_all = p3b.tile([128, KH, TPC], BF16, name="x2_all")  # 2.1 MB
        for m in range(KH):
            nc.vector.tensor_mul(x2_all[:, m, :], res2[:, m, :], rb3[:])
            if m == KH // 2 - 1:
                nc.sync.dma_start(g["ag_in_a"][:, :, :], x2_all[:, 0:KH // 2, :])
        nc.sync.dma_start(g["ag_in_b"][:, :, :], x2_all[:, KH // 2:KH, :])


def _phase4(nc, tc, g):
    """I-sharded gate/up projection + SwiGLU, full-S in one pass."""
    p4s, p4b = g["p4s"], g["p4b"]
    with (
        tc.tile_pool(name="p4psum", bufs=1, space="PSUM") as p4p,
    ):
        x2h_a = p4b.tile([128, KH // 2, S], BF16, name="x2h_a")  # 8.4 MB
        x2h_b = p4b.tile([128, KH // 2, S], BF16, name="x2h_b")  # 8.4 MB

        def stage(t, h, c):
            nc.sync.dma_start(t[:, :, c * TPC:(c + 1) * TPC],
                              g[f"ag_out_{h}"][c * 128:(c + 1) * 128, :, :])

        # first pair ahead of the second AllGather half so the first matmul's
        # inputs head the queue; then interleave (low c, low k) first
        stage(x2h_a, "a", 0)
        stage(x2h_a, "a", 1)
        g["emit_ag"]("b")
        stage(x2h_b, "b", 0)
        stage(x2h_b, "b", 1)
        for cp in range(1, NC // 2):
            for h, t in (("a", x2h_a), ("b", x2h_b)):
                for c in (2 * cp, 2 * cp + 1):
                    stage(t, h, c)

        def x2h_k(k, tcols):
            if k < KH // 2:
                return x2h_a[:, k, tcols]
            return x2h_b[:, k - KH // 2, tcols]

        issue_gu = g["issue_gu"]
        tiles = g["gu_tiles"]
        for m in range(MB_GU):
            if m + 1 < MB_GU and (m + 1) not in tiles:
                tiles[m + 1] = issue_gu(m + 1)
            gb, ub = tiles.pop(m)
            for tb in range(NB):
                tcols = slice(tb * 512, (tb + 1) * 512)
                g_ps = p4p.tile([128, 512], F32, name="g_ps", tag="g_ps", bufs=2)
                for k in range(KH):
                    nc.tensor.matmul(g_ps[:], gb[:, k, :], x2h_k(k, tcols),
                                     start=(k == 0), stop=(k == KH - 1))
                u_ps = p4p.tile([128, 512], F32, name="u_ps", tag="u_ps", bufs=2)
                for k in range(KH):
                    nc.tensor.matmul(u_ps[:], ub[:, k, :], x2h_k(k, tcols),
                                     start=(k == 0), stop=(k == KH - 1))
                sg = p4s.tile([128, 512], F32, name="sg", tag="sg", bufs=2)
                nc.scalar.activation(sg[:], g_ps[:], AF.Silu)
                hhh = p4s.tile([128, 512], BF16, name="hhh", tag="hhh", bufs=3)
                nc.vector.tensor_mul(hhh[:], sg[:], u_ps[:])
                nc.sync.dma_start(g["h_dram"][:, m, tcols], hhh[:])
            # stage this k-slice of h for phase 5 as soon as it lands
            if m < KI // 2:
                nc.sync.dma_start(g["hful_a"][:, m, :], g["h_dram"][:, m, :])
            if m == MB_GU - 2:
                g["issue_db"](0)
            elif m == MB_GU - 1:
                g["issue_db"](1)


def _phase5(nc, tc, g, with_collectives, rg):
    """Down projection (contraction over this core's I slice) + ReduceScatter."""
    with (
        tc.tile_pool(name="p5sbuf", bufs=1) as p5s,
        tc.tile_pool(name="p5big", bufs=1) as p5b,
        tc.tile_pool(name="p5psum", bufs=1, space="PSUM") as p5p,
    ):
        hful_b = p5b.tile([128, KI - KI // 2, S], BF16, name="hful_b")
        for k in range(KI // 2, KI):
            nc.sync.dma_start(hful_b[:, k - KI // 2, :], g["h_dram"][:, k, :])

        def hful_k(k, tcols):
            if k < KI // 2:
                return g["hful_a"][:, k, tcols]
            return hful_b[:, k - KI // 2, tcols]


        for r in range(8):
            for mi in range(KH // 8):
                m = r * (KH // 8) + mi
                if m + 2 < KH:
                    g["issue_db"](m + 2)
                db = g["db_tiles"][m]
                for tb in range(NB):
                    tcols = slice(tb * 512, (tb + 1) * 512)
                    d_ps = p5p.tile([128, 512], F32, name="d_ps", tag="d_ps", bufs=2)
                    for k in range(KI):
                        nc.tensor.matmul(d_ps[:], db[:, k, :], hful_k(k, tcols),
                                         start=(k == 0), stop=(k == KI - 1))
                    ot = p5s.tile([128, 512], BF16, name="ot", tag="ot", bufs=4)
                    nc.vector.tensor_copy(ot[:], d_ps[:])
                    nc.sync.dma_start(g[f"rs_in{r}"][mi * 128:(mi + 1) * 128, tcols],
                                      ot[:])
            if with_collectives:
                nc.gpsimd.collective_compute(
                    "ReduceScatter", mybir.AluOpType.add, replica_groups=rg,
                    ins=[g[f"rs_in{r}"].opt()], outs=[g[f"rs_out{r}"].opt()],
                )
            else:
                nc.sync.dma_start(g[f"rs_out{r}"][:, :],
                                  g[f"rs_in{r}"][0:H // NC // 8, :])
            nc.sync.dma_start(
                g["out_down"][r * 64:(r + 1) * 64, :], g[f"rs_out{r}"][:, :])


def build_program(with_collectives=True, stop_after=99):
    nc = bacc.Bacc("TRN2", target_bir_lowering=False, debug=False, num_devices=NC)

    g = {}
    g["hT"] = nc.dram_tensor("hT", [H, S], BF16, kind="ExternalInput")
    g["hT_slice"] = nc.dram_tensor("hT_slice", [128, KH, TPC], BF16, kind="ExternalInput")
    g["wqkv"] = nc.dram_tensor("wqkv", [128, KH, (QH + 2) * 128], BF16, kind="ExternalInput")
    g["wo"] = nc.dram_tensor("wo", [128, KH, KH, 128], BF16, kind="ExternalInput")
    g["wgu"] = nc.dram_tensor("wgu", [128, 2 * MB_GU, KH, 128], BF16, kind="ExternalInput")
    g["wdn"] = nc.dram_tensor("wdn", [128, KH, KI, 128], BF16, kind="ExternalInput")
    g["cosT"] = nc.dram_tensor("cosT", [128, S], BF16, kind="ExternalInput")
    g["sinT"] = nc.dram_tensor("sinT", [128, S], BF16, kind="ExternalInput")
    g["masks"] = nc.dram_tensor("masks", [128, 4, 512], BF16, kind="ExternalInput")

    g["res_out"] = nc.dram_tensor("res_out", [H, TPC], BF16, kind="ExternalOutput")
    g["out_down"] = nc.dram_tensor("out_down", [H // NC, S], BF16, kind="ExternalOutput")

    rg = [list(range(NC))]

    with tile.TileContext(nc) as tc:
        with (
            tc.tile_pool(name="consts", bufs=1) as consts,
            tc.tile_pool(name="dram", bufs=1, space="DRAM") as dram,
        ):
            for hh in range(QH):
                g[f"a2a_in{hh}"] = dram.tile([NC, 128, TPC], BF16, name=f"a2a_in{hh}")
                g[f"a2a_out{hh}"] = dram.tile([NC, 128, TPC], BF16, name=f"a2a_out{hh}")
            for h in ("a", "b"):
                g[f"ag_in_{h}"] = dram.tile([128, KH // 2, TPC], BF16, name=f"ag_in_{h}")
                g[f"ag_out_{h}"] = dram.tile([NC * 128, KH // 2, TPC], BF16,
                                             name=f"ag_out_{h}", addr_space="Shared")
            g["h_dram"] = dram.tile([128, KI, S], BF16, name="h_dram")
            for r in range(8):
                g[f"rs_in{r}"] = dram.tile([H // 8, S], BF16, name=f"rs_in{r}")
                g[f"rs_out{r}"] = dram.tile([H // NC // 8, S], BF16, name=f"rs_out{r}")

            ones32 = consts.tile([128, 1], F32, name="ones32")
            nc.gpsimd.memset(ones32[:], 1.0)
            g["ones32"] = ones32
            g["ones"] = consts.tile([128, 1], BF16, name="ones")
            nc.vector.tensor_copy(g["ones"][:], ones32[:])
            ident32 = consts.tile([128, 128], F32, name="ident32")
            make_identity(nc, ident32[:])
            g["ident"] = consts.tile([128, 128], BF16, name="ident")
            nc.vector.tensor_copy(g["ident"][:], ident32[:])
            g["epsb"] = consts.tile([1, 1], F32, name="epsb")
            nc.gpsimd.memset(g["epsb"][:], EPS)

            with (
                tc.tile_pool(name="p23stage", bufs=1) as p23,
                tc.tile_pool(name="wo_stream", bufs=1) as wop,
            ):
                g["asl"] = p23.tile([128, KH, TPC], BF16, name="asl")      # 2.1 MB
                g["hsl"] = p23.tile([128, KH, TPC], BF16, name="hsl")      # 2.1 MB

                g["wob_tiles"] = {}

                def issue_wob(m):
                    t = wop.tile([128, KH, 128], BF16, name=f"wob{m}",
                                 tag="wob", bufs=5)
                    nc.sync.dma_start(t[:], g["wo"][:, m, :, :])
                    g["wob_tiles"][m] = t

                g["issue_wob"] = issue_wob

                with (
                    tc.tile_pool(name="attn", bufs=1) as attn,
                    tc.tile_pool(name="p2work", bufs=1) as p2work,
                ):
                    g["mask_sb"] = attn.tile([128, 4, 512], BF16, name="mask_sb")
                    g["qT_sb"] = attn.tile([128, QH, S], BF16, name="qT_sb")      # 2 MB
                    g["kT_sb"] = attn.tile([128, S], BF16, name="kT_sb")          # 0.5 MB
                    g["v_tok"] = attn.tile([128, S // 128, 128], BF16, name="v_tok")
                    g["p2work"] = p2work

                    _phase1(nc, tc, g)
                    if stop_after >= 2:
                        _phase2(nc, tc, g, with_collectives, rg)

                if stop_after >= 3:
                    p4s_cm = tc.tile_pool(name="p4sbuf", bufs=1, side="right")
                    p4s = p4s_cm.__enter__()
                    g["p4s"] = p4s

                    def issue_gu(m):
                        gb = p4s.tile([128, KH, 128], BF16, name=f"gb{m}",
                                      tag="gb", bufs=2)
                        nc.sync.dma_start(gb[:], g["wgu"][:, m, :, :])
                        ub = p4s.tile([128, KH, 128], BF16, name=f"ub{m}",
                                      tag="ub", bufs=2)
                        nc.sync.dma_start(ub[:], g["wgu"][:, MB_GU + m, :, :])
                        return gb, ub

                    g["issue_gu"] = issue_gu
                    g["gu_tiles"] = {}

                    def prefetch_gu01():
                        g["gu_tiles"][0] = issue_gu(0)
                        g["gu_tiles"][1] = issue_gu(1)

                    g["prefetch_gu01"] = prefetch_gu01
                    _phase3(nc, tc, g, with_collectives, rg)

            if stop_after >= 4:
                with tc.tile_pool(name="mlp", bufs=1) as mlp:
                    g["hful_a"] = mlp.tile([128, KI // 2, S], BF16, name="hful_a")
                    g["db_tiles"] = {}

                    def issue_db(m):
                        t = mlp.tile([128, KI, 128], BF16, name=f"db{m}",
                                     tag="db", bufs=3)
                        nc.sync.dma_start(t[:], g["wdn"][:, m, :, :])
                        g["db_tiles"][m] = t

                    g["issue_db"] = issue_db

                    with (
                        tc.tile_pool(name="p4big", bufs=1) as p4b,
                    ):
                        g["p4b"] = p4b

                        def emit_ag(h):
                            if with_collectives:
                                nc.gpsimd.collective_compute(
                                    "AllGather", mybir.AluOpType.bypass,
                                    replica_groups=rg,
                                    ins=[g[f"ag_in_{h}"].opt()],
                                    outs=[g[f"ag_out_{h}"].opt()],
                                )
                            else:
                                nc.sync.dma_start(g[f"ag_out_{h}"][0:128, :, :],
                                                  g[f"ag_in_{h}"][:, :, :])

                        g["emit_ag"] = emit_ag
                        emit_ag("a")
                        _phase4(nc, tc, g)
                    if stop_after >= 5:
                        _phase5(nc, tc, g, with_collectives, rg)

            if stop_after >= 3:
                g.pop("prefetch_gu01", None)
                p4s_cm.__exit__(None, None, None)

    nc.finalize()
    return nc


_cached_nc = None


def _get_nc():
    global _cached_nc
    if _cached_nc is None:
        _cached_nc = build_program(with_collectives=True)
    return _cached_nc


def _host_prep(positions, hidden_states, w_qkv, w_o, w_gate_up, w_down, ln1_w, ln2_w):
    f32 = np.float32
    bf16 = ml_dtypes.bfloat16
    hidden = np.asarray(hidden_states, dtype=f32)[0]          # [S, H]
    hT = np.ascontiguousarray(hidden.T)                        # [H, S]
    pos = np.asarray(positions).astype(f32)[0]                 # [S]

    half = HD // 2
    inv_freq = (1.0 / (f32(THETA) ** (np.arange(0, half, dtype=f32) / f32(half)))).astype(f32)
    ang = pos[:, None] * inv_freq[None, :]                     # [S, 64] fp32
    cos_half = np.cos(ang).astype(f32).T                       # [64, S]
    sin_half = np.sin(ang).astype(f32).T
    cosT_np = np.concatenate([cos_half, cos_half], axis=0).astype(bf16)  # [128, S]
    sinT_np = np.concatenate([sin_half, sin_half], axis=0).astype(bf16)
    cosT_np = np.ascontiguousarray(cosT_np)
    sinT_np = np.ascontiguousarray(sinT_np)

    w_qkv_f = np.asarray(w_qkv, dtype=f32) * np.asarray(ln1_w, dtype=f32)[:, None]
    w_gu_f = np.asarray(w_gate_up, dtype=f32) * np.asarray(ln2_w, dtype=f32)[:, None]
    w_o_f = np.asarray(w_o, dtype=f32).reshape(KH, 128, KH, 128).transpose(1, 2, 0, 3)
    # k axis reordered head-major to match asl staging: knew = hh*8+r holds
    # global head r*QH+hh
    perm = [(knew % 8) * QH + knew // 8 for knew in range(KH)]
    w_o_f = np.ascontiguousarray(w_o_f[:, :, perm, :]).astype(bf16)
    w_dn_f = np.asarray(w_down, dtype=f32)

    kk = np.arange(128)[:, None, None]
    jj = np.arange(4)[None, :, None]
    qq = np.arange(512)[None, None, :]
    masks_np = np.ascontiguousarray((qq >= kk + 128 * jj).astype(bf16))  # [128, 4, 512]

    hT_bf = np.ascontiguousarray(hT.astype(bf16))

    in_maps = []
    for c in range(NC):
        q_cols = w_qkv_f[:, c * QH * HD:(c + 1) * QH * HD]
        k_col = w_qkv_f[:, NQ * HD + c * HD: NQ * HD + (c + 1) * HD]
        v_col = w_qkv_f[:, (NQ + NKV) * HD + c * HD: (NQ + NKV) * HD + (c + 1) * HD]
        wqkv_c = np.concatenate([q_cols, k_col, v_col], axis=1)
        wqkv_c = np.ascontiguousarray(
            wqkv_c.reshape(KH, 128, (QH + 2) * 128).transpose(1, 0, 2)).astype(bf16)
        wgu_c = np.concatenate(
            [w_gu_f[:, c * IPC:(c + 1) * IPC],
             w_gu_f[:, I + c * IPC: I + (c + 1) * IPC]], axis=1)
        wgu_c = np.ascontiguousarray(
            wgu_c.reshape(KH, 128, 2 * MB_GU, 128).transpose(1, 2, 0, 3)).astype(bf16)
        wdn_c = np.ascontiguousarray(
            w_dn_f[c * IPC:(c + 1) * IPC, :].reshape(KI, 128, KH, 128)
            .transpose(1, 2, 0, 3)).astype(bf16)
        hT_slice_c = np.ascontiguousarray(
            hT[:, c * TPC:(c + 1) * TPC].reshape(KH, 128, TPC)
            .transpose(1, 0, 2)).astype(ml_dtypes.bfloat16)
        in_maps.append({
            "hT": hT_bf,
            "hT_slice": hT_slice_c,
            "wqkv": np.ascontiguousarray(wqkv_c),
            "wo": w_o_f,
            "wgu": np.ascontiguousarray(wgu_c),
            "wdn": np.ascontiguousarray(wdn_c),
            "cosT": cosT_np,
            "sinT": sinT_np,
            "masks": masks_np,
        })
    return in_maps


def kernel(**inputs):
    in_maps = _host_prep(**inputs)
    nc = _get_nc()
    res = run_bass_kernel_spmd(nc, in_maps, core_ids=list(range(NC)))
    results = res.results

    outT = np.empty((H, S), np.float32)
    for c in range(NC):
        od = np.asarray(results[c]["out_down"]).astype(np.float32)  # [512, S]
        for r in range(8):
            outT[512 * r + 64 * c: 512 * r + 64 * (c + 1)] = od[64 * r:64 * (r + 1)]
    resT = np.concatenate(
        [np.asarray(results[c]["res_out"]).astype(np.float32) for c in range(NC)],
        axis=1)                                                      # [H, S]
    out = np.ascontiguousarray(outT.T).reshape(1, S, H).astype(np.float32)
    residual = np.ascontiguousarray(resT.T).reshape(1, S, H).astype(np.float32)
    return out, residual


# revision 92
# speedup vs baseline: 1.0022x; 1.0022x over previous
"""Bamba attention decoder layer on 8 Trainium2 NeuronCores.

Sharding: tensor-parallel attention (4 q heads + 1 kv head per core),
AllToAll of attention context (delivers each core its token slice at a static
address), token-sliced o_proj + fused add/rmsnorm, AllGather of normed
activations, I-sharded SwiGLU MLP (1792 cols/core), ReduceScatter of
down-proj partials.

All matmul operands are bf16 (fp32 PSUM accumulation and fp32 rmsnorm
stats); the rmsnorm sum-of-squares accumulates on the otherwise-idle GpSimd
engine (one f32 ones-matmul per block for the partition reduction); the
softmax sum-of-exp accumulates on the PE via a ones-matmul instead of DVE
adds; causal masks are bf16 multiplies and diagonal score tiles are narrowed
to their causally live q-range. DMA issue order is engineered around the
FIFO sync queue: the first QKV matmul starts after ~0.6MB of traffic,
o_proj weights and attention-context slices stream in during attention (one
transposed-view DMA per head), the first gate/up weight pairs are emitted
late in the o_proj loop so their transfers fill the rmsnorm2 window, the
AllGather of normed activations is split in two k-halves with the first
gathered activation slices queued between them, gate/up/down weight streams
are ring-buffered with prefetch, and the first half of the SwiGLU
intermediate restages into SBUF while phase 4 still runs.
"""

import numpy as np
import ml_dtypes

import concourse.bacc as bacc
import concourse.mybir as mybir
import concourse.tile as tile
from concourse.bass_utils import run_bass_kernel_spmd
from concourse.masks import make_identity

NC = 8
S = 2048
H = 4096
HD = 128
NQ = 32
NKV = 8
I = 14336
QH = NQ // NC        # q heads per core = 4
IPC = I // NC        # intermediate cols per core = 1792
TPC = S // NC        # tokens per core = 256
EPS = 1e-5
THETA = 10000.0
SCALE = HD ** -0.5

F32 = mybir.dt.float32
BF16 = mybir.dt.bfloat16

KH = H // 128        # 32 k-tiles over H
NB = S // 512        # 4 token blocks of 512
MB_GU = IPC // 128   # 14 m tiles for gate (and for up)
KI = IPC // 128      # 14 k tiles over I per core

AF = mybir.ActivationFunctionType


def _phase1(nc, tc, g):
    """QKV matmul + rmsnorm1 stats + rope. Fills qT_sb/kT_sb/v_tok."""
    with (
        tc.tile_pool(name="p1sbuf", bufs=1) as p1s,
        tc.tile_pool(name="p1w", bufs=1) as p1w,
        tc.tile_pool(name="p1psum", bufs=1, space="PSUM") as p1p,
    ):
        wq_sb = p1w.tile([128, KH, (QH + 2) * 128], BF16, name="wq_sb")  # 6.3 MB
        nc.sync.dma_start(wq_sb[:, 0:2, :], g["wqkv"][:, 0:2, :])
        cos_sb = p1w.tile([128, S], BF16, name="cos_sb")
        sin_sb = p1w.tile([128, S], BF16, name="sin_sb")

        for nb in range(NB):
            ncols = slice(nb * 512, (nb + 1) * 512)
            st_ps = p1p.tile([1, 512], F32, name="st_ps", tag="st_ps")
            acc = p1s.tile([128, 512], F32, name="acc", tag="acc", bufs=1)
            nc.gpsimd.memset(acc[:], 0.0)
            mm_ps = []
            for m in range(QH + 2):
                t = p1p.tile([128, 512], F32, name=f"qkv_ps{m}", tag=f"qkv_ps{m}")
                mm_ps.append(t)
            for k in range(KH):
                if nb == 0:
                    # stagger the remaining weight chunks + rope tables behind
                    # the hb stream so the first matmuls start early
                    if k == 1:
                        nc.sync.dma_start(wq_sb[:, 2:8, :], g["wqkv"][:, 2:8, :])
                    elif k == 4:
                        nc.sync.dma_start(wq_sb[:, 8:16, :], g["wqkv"][:, 8:16, :])
                    elif k == 10:
                        nc.sync.dma_start(wq_sb[:, 16:24, :], g["wqkv"][:, 16:24, :])
                    elif k == 16:
                        nc.sync.dma_start(wq_sb[:, 24:32, :], g["wqkv"][:, 24:32, :])
                    elif k == 22:
                        nc.sync.dma_start(cos_sb[:], g["cosT"][:, :])
                    elif k == 26:
                        nc.sync.dma_start(sin_sb[:], g["sinT"][:, :])
                    elif k == 30:
                        nc.sync.dma_start(g["mask_sb"][:], g["masks"][:, :, :])
                hb = p1s.tile([128, 512], BF16, name="hb", tag="hb", bufs=4)
                nc.sync.dma_start(hb[:], g["hT"][k * 128:(k + 1) * 128, ncols])
                sq = p1s.tile([128, 512], BF16, name="sq", tag="sq", bufs=3)
                nc.vector.tensor_mul(sq[:], hb[:], hb[:])
                # accumulate the sum-of-squares on the (otherwise idle) Pool
                # engine; the cross-partition reduction happens once per block
                nc.gpsimd.tensor_add(acc[:], acc[:], sq[:])
                for m in range(QH + 2):
                    nc.tensor.matmul(
                        mm_ps[m][:], wq_sb[:, k, m * 128:(m + 1) * 128], hb[:],
                        start=(k == 0), stop=(k == KH - 1),
                    )
            # rmsnorm stats for this token block
            nc.tensor.matmul(st_ps[:], g["ones32"][:], acc[:], start=True, stop=True)
            std_row = p1s.tile([1, 512], F32, name="std_row", tag="std_row")
            nc.scalar.activation(std_row[:], st_ps[:], AF.Sqrt,
                                 bias=g["epsb"][:], scale=1.0 / H)
            rstd = p1s.tile([1, 512], F32, name="rstd", tag="rstd")
            nc.vector.reciprocal(rstd[:], std_row[:])
            rb32 = p1s.tile([128, 512], F32, name="rb32", tag="rb32")
            nc.gpsimd.partition_broadcast(rb32[:], rstd[:])
            rstdb = p1s.tile([1, 512], BF16, name="rstdb", tag="rstdb")
            nc.vector.tensor_copy(rstdb[:], rstd[:])
            rbb = p1s.tile([128, 512], BF16, name="rbb", tag="rbb")
            nc.gpsimd.partition_broadcast(rbb[:], rstdb[:])
            # 1/rms folded into the rope tables (per-token column scale)
            cos_s = p1s.tile([128, 512], BF16, name="cos_s", tag="cos_s")
            nc.vector.tensor_mul(cos_s[:], cos_sb[:, ncols], rbb[:])
            sin_s = p1s.tile([128, 512], BF16, name="sin_s", tag="sin_s")
            nc.vector.tensor_mul(sin_s[:], sin_sb[:, ncols], rbb[:])
            # evacuate the 5 rope-bound psums so the PE can start the next
            # token block while rope runs from SBUF
            qkc = p1s.tile([128, QH + 1, 512], BF16, name="qkc", tag="qkc", bufs=2)
            for m in range(QH + 1):
                # alternate engines so the psum evacuation chain halves
                if m % 2 == 0:
                    nc.scalar.copy(qkc[:, m, :], mm_ps[m][:])
                else:
                    nc.vector.tensor_copy(qkc[:, m, :], mm_ps[m][:])
            for m in range(QH + 1):
                if m < QH:
                    d0 = g["qT_sb"][0:64, m, ncols]
                    d1 = g["qT_sb"][64:128, m, ncols]
                else:
                    d0 = g["kT_sb"][0:64, ncols]
                    d1 = g["kT_sb"][64:128, ncols]
                t0 = p1s.tile([64, 512], BF16, name="t0", tag="t0")
                nc.vector.tensor_mul(t0[:], qkc[0:64, m, :], cos_s[0:64, :])
                t1 = p1s.tile([64, 512], BF16, name="t1", tag="t1")
                nc.vector.tensor_mul(t1[:], qkc[64:128, m, :], sin_s[64:128, :])
                nc.vector.tensor_sub(d0, t0[:], t1[:])
                t2 = p1s.tile([64, 512], BF16, name="t2", tag="t0")
                nc.vector.tensor_mul(t2[:], qkc[64:128, m, :], cos_s[64:128, :])
                t3 = p1s.tile([64, 512], BF16, name="t3", tag="t1")
                nc.vector.tensor_mul(t3[:], qkc[0:64, m, :], sin_s[0:64, :])
                nc.vector.tensor_add(d1, t2[:], t3[:])
            vtmp = p1s.tile([128, 512], BF16, name="vtmp", tag="vtmp")
            nc.vector.tensor_mul(vtmp[:], mm_ps[QH + 1][:], rb32[:])
            tp = p1p.tile([128, 4, 128], BF16, name="tp", tag="tp")
            for j in range(4):
                nc.tensor.transpose(tp[:, j, :], vtmp[:, j * 128:(j + 1) * 128],
                                    g["ident"][:])
            nc.vector.tensor_copy(g["v_tok"][:, nb * 4:(nb + 1) * 4, :], tp[:])


def _phase2(nc, tc, g, with_collectives, rg):
    """Causal GQA attention. Sum-of-exp accumulated on PE via ones-matmul."""
    p2s = g["p2work"]
    with (
        tc.tile_pool(name="p2psum", bufs=1, space="PSUM") as p2p,
    ):
        for hh in range(QH):
            for qb in range(NB):
                qcols = slice(qb * 512, (qb + 1) * 512)
                nkt = 4 * qb + 4
                att_ps = p2p.tile([128, 512], F32, name="att_ps", tag="att_ps", bufs=2)
                sums_ps = p2p.tile([1, 512], F32, name="sums_ps", tag="sums_ps", bufs=2)
                for kt in range(nkt):
                    j = kt - 4 * qb
                    # diagonal tile j covers only q >= 128*j within the block
                    lo = 128 * j if j > 0 else 0
                    qs = slice(qb * 512 + lo, (qb + 1) * 512)
                    s_ps = p2p.tile([128, 512], F32, name="s_ps", tag="s_ps", bufs=4)
                    nc.tensor.matmul(
                        s_ps[:, lo:512], g["kT_sb"][:, kt * 128:(kt + 1) * 128],
                        g["qT_sb"][:, hh, qs], start=True, stop=True,
                    )
                    e = p2s.tile([128, 512], BF16, name="e", tag="e", bufs=8)
                    nc.scalar.activation(e[:, lo:512], s_ps[:, lo:512],
                                         AF.Exp, scale=SCALE)
                    if j >= 0:
                        nc.vector.tensor_mul(e[:, lo:512], e[:, lo:512],
                                             g["mask_sb"][:, j, lo:512])
                    nc.tensor.matmul(sums_ps[:, lo:512], g["ones"][:], e[:, lo:512],
                                     start=(kt == 0), stop=(kt == nkt - 1))
                    nc.tensor.matmul(att_ps[:, lo:512], g["v_tok"][:, kt, :],
                                     e[:, lo:512],
                                     start=(kt == 0), stop=(kt == nkt - 1))
                recip = p2s.tile([1, 512], F32, name="recip", tag="recip", bufs=2)
                nc.vector.reciprocal(recip[:], sums_ps[:])
                rb2 = p2s.tile([128, 512], F32, name="rb2", tag="rb2", bufs=2)
                nc.gpsimd.partition_broadcast(rb2[:], recip[:])
                anorm = p2s.tile([128, 512], BF16, name="anorm", tag="anorm", bufs=2)
                nc.vector.tensor_mul(anorm[:], att_ps[:], rb2[:])
                for half in range(2):
                    dst_core = qb * 2 + half
                    nc.sync.dma_start(
                        g[f"a2a_in{hh}"][dst_core, :, :],
                        anorm[:, half * 256:(half + 1) * 256],
                    )
            # ship this head's context while the next head computes
            if with_collectives:
                nc.gpsimd.collective_compute(
                    "AllToAll", mybir.AluOpType.bypass, replica_groups=rg,
                    ins=[g[f"a2a_in{hh}"].opt()], outs=[g[f"a2a_out{hh}"].opt()],
                )
            else:
                nc.sync.dma_start(g[f"a2a_out{hh}"][:, :, :], g[f"a2a_in{hh}"][:, :, :])
            # stage this head's o_proj input slices as they land (one DMA,
            # transposed view: [r, p, t] -> [p, r, t])
            nc.sync.dma_start(
                g["asl"][:, hh * 8:(hh + 1) * 8, :],
                g[f"a2a_out{hh}"][:, :, :].transpose([1, 0, 2]),
            )
            # opportunistic prefetch for phase 3
            if hh == 0:
                for kq in range(4):
                    nc.sync.dma_start(g["hsl"][:, kq * 8:(kq + 1) * 8, :],
                                      g["hT_slice"][:, kq * 8:(kq + 1) * 8, :])
            elif hh == 1:
                g["issue_wob"](0)
                g["issue_wob"](1)
            elif hh == 2:
                g["issue_wob"](2)
            elif hh == 3:
                g["issue_wob"](3)


def _phase3(nc, tc, g, with_collectives, rg):
    """Token-sliced o_proj + residual add + rmsnorm2 + AllGather of x2."""
    with (
        tc.tile_pool(name="p3sbuf", bufs=1) as p3s,
        tc.tile_pool(name="p3big", bufs=1) as p3b,
        tc.tile_pool(name="p3psum", bufs=1, space="PSUM") as p3p,
    ):
        res2 = p3b.tile([128, KH, TPC], BF16, name="res2")  # 2 MB
        st2_ps = p3p.tile([1, TPC], F32, name="st2_ps", tag="st2_ps")
        acc2 = p3s.tile([128, TPC], F32, name="acc2", tag="acc2", bufs=1)
        nc.gpsimd.memset(acc2[:], 0.0)
        # asl k index is head-major (hh*8+r): head 3's A2A lands last
        for m in range(KH):
            if m + 4 < KH:
                g["issue_wob"](m + 4)
            if m == 28 and "prefetch_gu01" in g:
                # first two gate/up weight pairs; emitted here (after all wob
                # issues) so their transfers fill the stats/x2 window and the
                # AllGather chain heads the queue at the phase boundary
                g["prefetch_gu01"]()
            wob = g["wob_tiles"][m]
            o_ps = p3p.tile([128, TPC], F32, name="o_ps", tag="o_ps", bufs=4)
            for k in range(KH):
                nc.tensor.matmul(o_ps[:], wob[:, k, :], g["asl"][:, k, :],
                                 start=(k == 0), stop=(k == KH - 1))
            hslm = p3s.tile([128, TPC], F32, name="hslm", tag="hslm", bufs=2)
            nc.vector.tensor_copy(hslm[:], g["hsl"][:, m, :])
            nc.vector.tensor_add(res2[:, m, :], o_ps[:], hslm[:])
            nc.sync.dma_start(g["res_out"][m * 128:(m + 1) * 128, :], res2[:, m, :])
            sq2 = p3s.tile([128, TPC], BF16, name="sq2", tag="sq2", bufs=2)
            nc.vector.tensor_mul(sq2[:], res2[:, m, :], res2[:, m, :])
            nc.gpsimd.tensor_add(acc2[:], acc2[:], sq2[:])
        nc.tensor.matmul(st2_ps[:], g["ones32"][:], acc2[:], start=True, stop=True)
        std2 = p3s.tile([1, TPC], F32, name="std2", tag="std2")
        nc.scalar.activation(std2[:], st2_ps[:], AF.Sqrt, bias=g["epsb"][:],
                             scale=1.0 / H)
        rstd2 = p3s.tile([1, TPC], F32, name="rstd2", tag="rstd2")
        nc.vector.reciprocal(rstd2[:], std2[:])
        rstd2b = p3s.tile([1, TPC], BF16, name="rstd2b", tag="rstd2b")
        nc.vector.tensor_copy(rstd2b[:], rstd2[:])
        rb3 = p3s.tile([128, TPC], BF16, name="rb3", tag="rb3")
        nc.gpsimd.partition_broadcast(rb3[:], rstd2b[:])
        x2_all = p3b.tile([128, KH, TPC], BF16, name="x2_all")  # 2.1 MB
        for m in range(KH):
            nc.vector.tensor_mul(x2_all[:, m, :], res2[:, m, :], rb3[:])
            if m == KH // 2 - 1:
                nc.sync.dma_start(g["ag_in_a"][:, :, :], x2_all[:, 0:KH // 2, :])
        nc.sync.dma_start(g["ag_in_b"][:, :, :], x2_all[:, KH // 2:KH, :])


def _phase4(nc, tc, g):
    """I-sharded gate/up projection + SwiGLU, full-S in one pass."""
    p4s, p4b = g["p4s"], g["p4b"]
    with (
        tc.tile_pool(name="p4psum", bufs=1, space="PSUM") as p4p,
    ):
        x2h_a = p4b.tile([128, KH // 2, S], BF16, name="x2h_a")  # 8.4 MB
        x2h_b = p4b.tile([128, KH // 2, S], BF16, name="x2h_b")  # 8.4 MB

        def stage(t, h, c):
            nc.sync.dma_start(t[:, :, c * TPC:(c + 1) * TPC],
                              g[f"ag_out_{h}"][c * 128:(c + 1) * 128, :, :])

        # first pair ahead of the second AllGather half so the first matmul's
        # inputs head the queue; then interleave (low c, low k) first
        stage(x2h_a, "a", 0)
        stage(x2h_a, "a", 1)
        g["emit_ag"]("b")
        stage(x2h_b, "b", 0)
        stage(x2h_b, "b", 1)
        for cp in range(1, NC // 2):
            for h, t in (("a", x2h_a), ("b", x2h_b)):
                for c in (2 * cp, 2 * cp + 1):
                    stage(t, h, c)

        def x2h_k(k, tcols):
            if k < KH // 2:
                return x2h_a[:, k, tcols]
            return x2h_b[:, k - KH // 2, tcols]

        issue_gu = g["issue_gu"]
        tiles = g["gu_tiles"]
        for m in range(MB_GU):
            if m + 1 < MB_GU and (m + 1) not in tiles:
                tiles[m + 1] = issue_gu(m + 1)
            gb, ub = tiles.pop(m)
            for tb in range(NB):
                tcols = slice(tb * 512, (tb + 1) * 512)
                g_ps = p4p.tile([128, 512], F32, name="g_ps", tag="g_ps", bufs=2)
                for k in range(KH):
                    nc.tensor.matmul(g_ps[:], gb[:, k, :], x2h_k(k, tcols),
                                     start=(k == 0), stop=(k == KH - 1))
                u_ps = p4p.tile([128, 512], F32, name="u_ps", tag="u_ps", bufs=2)
                for k in range(KH):
                    nc.tensor.matmul(u_ps[:], ub[:, k, :], x2h_k(k, tcols),
                                     start=(k == 0), stop=(k == KH - 1))
                sg = p4s.tile([128, 512], F32, name="sg", tag="sg", bufs=2)
                nc.scalar.activation(sg[:], g_ps[:], AF.Silu)
                hhh = p4s.tile([128, 512], BF16, name="hhh", tag="hhh", bufs=3)
                nc.vector.tensor_mul(hhh[:], sg[:], u_ps[:])
                nc.sync.dma_start(g["h_dram"][:, m, tcols], hhh[:])
            # stage this k-slice of h for phase 5 as soon as it lands
            if m < KI // 2:
                nc.sync.dma_start(g["hful_a"][:, m, :], g["h_dram"][:, m, :])
            if m == MB_GU - 2:
                g["issue_db"](0)
            elif m == MB_GU - 1:
                g["issue_db"](1)


def _phase5(nc, tc, g, with_collectives, rg):
    """Down projection (contraction over this core's I slice) + ReduceScatter."""
    with (
        tc.tile_pool(name="p5sbuf", bufs=1) as p5s,
        tc.tile_pool(name="p5big", bufs=1) as p5b,
        tc.tile_pool(name="p5psum", bufs=1, space="PSUM") as p5p,
    ):
        hful_b = p5b.tile([128, KI - KI // 2, S], BF16, name="hful_b")
        for k in range(KI // 2, KI):
            nc.sync.dma_start(hful_b[:, k - KI // 2, :], g["h_dram"][:, k, :])

        def hful_k(k, tcols):
            if k < KI // 2:
                return g["hful_a"][:, k, tcols]
            return hful_b[:, k - KI // 2, tcols]


        for r in range(8):
            for mi in range(KH // 8):
                m = r * (KH // 8) + mi
                if m + 2 < KH:
                    g["issue_db"](m + 2)
                db = g["db_tiles"][m]
                for tb in range(NB):
                    tcols = slice(tb * 512, (tb + 1) * 512)
                    d_ps = p5p.tile([128, 512], F32, name="d_ps", tag="d_ps", bufs=2)
                    for k in range(KI):
                        nc.tensor.matmul(d_ps[:], db[:, k, :], hful_k(k, tcols),
                                         start=(k == 0), stop=(k == KI - 1))
                    ot = p5s.tile([128, 512], BF16, name="ot", tag="ot", bufs=4)
                    nc.vector.tensor_copy(ot[:], d_ps[:])
                    nc.sync.dma_start(g[f"rs_in{r}"][mi * 128:(mi + 1) * 128, tcols],
                                      ot[:])
            if with_collectives:
                nc.gpsimd.collective_compute(
                    "ReduceScatter", mybir.AluOpType.add, replica_groups=rg,
                    ins=[g[f"rs_in{r}"].opt()], outs=[g[f"rs_out{r}"].opt()],
                )
            else:
                nc.sync.dma_start(g[f"rs_out{r}"][:, :],
                                  g[f"rs_in{r}"][0:H // NC // 8, :])
            nc.sync.dma_start(
                g["out_down"][r * 64:(r + 1) * 64, :], g[f"rs_out{r}"][:, :])


def build_program(with_collectives=True, stop_after=99):
    nc = bacc.Bacc("TRN2", target_bir_lowering=False, debug=False, num_devices=NC)

    g = {}
    g["hT"] = nc.dram_tensor("hT", [H, S], BF16, kind="ExternalInput")
    g["hT_slice"] = nc.dram_tensor("hT_slice", [128, KH, TPC], BF16, kind="ExternalInput")
    g["wqkv"] = nc.dram_tensor("wqkv", [128, KH, (QH + 2) * 128], BF16, kind="ExternalInput")
    g["wo"] = nc.dram_tensor("wo", [128, KH, KH, 128], BF16, kind="ExternalInput")
    g["wgu"] = nc.dram_tensor("wgu", [128, 2 * MB_GU, KH, 128], BF16, kind="ExternalInput")
    g["wdn"] = nc.dram_tensor("wdn", [128, KH, KI, 128], BF16, kind="ExternalInput")
    g["cosT"] = nc.dram_tensor("cosT", [128, S], BF16, kind="ExternalInput")
    g["sinT"] = nc.dram_tensor("sinT", [128, S], BF16, kind="ExternalInput")
    g["masks"] = nc.dram_tensor("masks", [128, 4, 512], BF16, kind="ExternalInput")

    g["res_out"] = nc.dram_tensor("res_out", [H, TPC], BF16, kind="ExternalOutput")
    g["out_down"] = nc.dram_tensor("out_down", [H // NC, S], BF16, kind="ExternalOutput")

    rg = [list(range(NC))]

    with tile.TileContext(nc) as tc:
        with (
            tc.tile_pool(name="consts", bufs=1) as consts,
            tc.tile_pool(name="dram", bufs=1, space="DRAM") as dram,
        ):
            for hh in range(QH):
                g[f"a2a_in{hh}"] = dram.tile([NC, 128, TPC], BF16, name=f"a2a_in{hh}")
                g[f"a2a_out{hh}"] = dram.tile([NC, 128, TPC], BF16, name=f"a2a_out{hh}")
            for h in ("a", "b"):
                g[f"ag_in_{h}"] = dram.tile([128, KH // 2, TPC], BF16, name=f"ag_in_{h}")
                g[f"ag_out_{h}"] = dram.tile([NC * 128, KH // 2, TPC], BF16,
                                             name=f"ag_out_{h}", addr_space="Shared")
            g["h_dram"] = dram.tile([128, KI, S], BF16, name="h_dram")
            for r in range(8):
                g[f"rs_in{r}"] = dram.tile([H // 8, S], BF16, name=f"rs_in{r}")
                g[f"rs_out{r}"] = dram.tile([H // NC // 8, S], BF16, name=f"rs_out{r}")

            ones32 = consts.tile([128, 1], F32, name="ones32")
            nc.gpsimd.memset(ones32[:], 1.0)
            g["ones32"] = ones32
            g["ones"] = consts.tile([128, 1], BF16, name="ones")
            nc.vector.tensor_copy(g["ones"][:], ones32[:])
            ident32 = consts.tile([128, 128], F32, name="ident32")
            make_identity(nc, ident32[:])
            g["ident"] = consts.tile([128, 128], BF16, name="ident")
            nc.vector.tensor_copy(g["ident"][:], ident32[:])
            g["epsb"] = consts.tile([1, 1], F32, name="epsb")
            nc.gpsimd.memset(g["epsb"][:], EPS)

            with (
                tc.tile_pool(name="p23stage", bufs=1) as p23,
                tc.tile_pool(name="wo_stream", bufs=1) as wop,
            ):
                g["asl"] = p23.tile([128, KH, TPC], BF16, name="asl")      # 2.1 MB
                g["hsl"] = p23.tile([128, KH, TPC], BF16, name="hsl")      # 2.1 MB

                g["wob_tiles"] = {}

                def issue_wob(m):
                    t = wop.tile([128, KH, 128], BF16, name=f"wob{m}",
                                 tag="wob", bufs=5)
                    nc.sync.dma_start(t[:], g["wo"][:, m, :, :])
                    g["wob_tiles"][m] = t

                g["issue_wob"] = issue_wob

                with (
                    tc.tile_pool(name="attn", bufs=1) as attn,
                    tc.tile_pool(name="p2work", bufs=1) as p2work,
                ):
                    g["mask_sb"] = attn.tile([128, 4, 512], BF16, name="mask_sb")
                    g["qT_sb"] = attn.tile([128, QH, S], BF16, name="qT_sb")      # 2 MB
                    g["kT_sb"] = attn.tile([128, S], BF16, name="kT_sb")          # 0.5 MB
                    g["v_tok"] = attn.tile([128, S // 128, 128], BF16, name="v_tok")
                    g["p2work"] = p2work

                    _phase1(nc, tc, g)
                    if stop_after >= 2:
                        _phase2(nc, tc, g, with_collectives, rg)

                if stop_after >= 3:
                    p4s_cm = tc.tile_pool(name="p4sbuf", bufs=1, side="right")
                    p4s = p4s_cm.__enter__()
                    g["p4s"] = p4s

                    def issue_gu(m):
                        gb = p4s.tile([128, KH, 128], BF16, name=f"gb{m}",
                                      tag="gb", bufs=2)
                        nc.sync.dma_start(gb[:], g["wgu"][:, m, :, :])
                        ub = p4s.tile([128, KH, 128], BF16, name=f"ub{m}",
                                      tag="ub", bufs=2)
                        nc.sync.dma_start(ub[:], g["wgu"][:, MB_GU + m, :, :])
                        return gb, ub

                    g["issue_gu"] = issue_gu
                    g["gu_tiles"] = {}

                    def prefetch_gu01():
                        g["gu_tiles"][0] = issue_gu(0)
                        g["gu_tiles"][1] = issue_gu(1)

                    g["prefetch_gu01"] = prefetch_gu01
                    _phase3(nc, tc, g, with_collectives, rg)

            if stop_after >= 4:
                with tc.tile_pool(name="mlp", bufs=1) as mlp:
                    g["hful_a"] = mlp.tile([128, KI // 2, S], BF16, name="hful_a")
                    g["db_tiles"] = {}

                    def issue_db(m):
                        t = mlp.tile([128, KI, 128], BF16, name=f"db{m}",
                                     tag="db", bufs=3)
                        nc.sync.dma_start(t[:], g["wdn"][:, m, :, :])
                        g["db_tiles"][m] = t

                    g["issue_db"] = issue_db

                    with (
                        tc.tile_pool(name="p4big", bufs=1) as p4b,
                    ):
                        g["p4b"] = p4b

                        def emit_ag(h):
                            if with_collectives:
                                nc.gpsimd.collective_compute(
                                    "AllGather", mybir.AluOpType.bypass,
                                    replica_groups=rg,
                                    ins=[g[f"ag_in_{h}"].opt()],
                                    outs=[g[f"ag_out_{h}"].opt()],
                                )
                            else:
                                nc.sync.dma_start(g[f"ag_out_{h}"][0:128, :, :],
                                                  g[f"ag_in_{h}"][:, :, :])

                        g["emit_ag"] = emit_ag
                        emit_ag("a")
                        _phase4(nc, tc, g)
                    if stop_after >= 5:
                        _phase5(nc, tc, g, with_collectives, rg)

            if stop_after >= 3:
                g.pop("prefetch_gu01", None)
                p4s_cm.__exit__(None, None, None)

    nc.finalize()
    return nc


_cached_nc = None


def _get_nc():
    global _cached_nc
    if _cached_nc is None:
        _cached_nc = build_program(with_collectives=True)
    return _cached_nc


def _host_prep(positions, hidden_states, w_qkv, w_o, w_gate_up, w_down, ln1_w, ln2_w):
    f32 = np.float32
    bf16 = ml_dtypes.bfloat16
    hidden = np.asarray(hidden_states, dtype=f32)[0]          # [S, H]
    hT = np.ascontiguousarray(hidden.T)                        # [H, S]
    pos = np.asarray(positions).astype(f32)[0]                 # [S]

    half = HD // 2
    inv_freq = (1.0 / (f32(THETA) ** (np.arange(0, half, dtype=f32) / f32(half)))).astype(f32)
    ang = pos[:, None] * inv_freq[None, :]                     # [S, 64] fp32
    cos_half = np.cos(ang).astype(f32).T                       # [64, S]
    sin_half = np.sin(ang).astype(f32).T
    cosT_np = np.concatenate([cos_half, cos_half], axis=0).astype(bf16)  # [128, S]
    sinT_np = np.concatenate([sin_half, sin_half], axis=0).astype(bf16)
    cosT_np = np.ascontiguousarray(cosT_np)
    sinT_np = np.ascontiguousarray(sinT_np)

    w_qkv_f = np.asarray(w_qkv, dtype=f32) * np.asarray(ln1_w, dtype=f32)[:, None]
    w_gu_f = np.asarray(w_gate_up, dtype=f32) * np.asarray(ln2_w, dtype=f32)[:, None]
    w_o_f = np.asarray(w_o, dtype=f32).reshape(KH, 128, KH, 128).transpose(1, 2, 0, 3)
    # k axis reordered head-major to match asl staging: knew = hh*8+r holds
    # global head r*QH+hh
    perm = [(knew % 8) * QH + knew // 8 for knew in range(KH)]
    w_o_f = np.ascontiguousarray(w_o_f[:, :, perm, :]).astype(bf16)
    w_dn_f = np.asarray(w_down, dtype=f32)

    kk = np.arange(128)[:, None, None]
    jj = np.arange(4)[None, :, None]
    qq = np.arange(512)[None, None, :]
    masks_np = np.ascontiguousarray((qq >= kk + 128 * jj).astype(bf16))  # [128, 4, 512]

    hT_bf = np.ascontiguousarray(hT.astype(bf16))

    in_maps = []
    for c in range(NC):
        q_cols = w_qkv_f[:, c * QH * HD:(c + 1) * QH * HD]
        k_col = w_qkv_f[:, NQ * HD + c * HD: NQ * HD + (c + 1) * HD]
        v_col = w_qkv_f[:, (NQ + NKV) * HD + c * HD: (NQ + NKV) * HD + (c + 1) * HD]
        wqkv_c = np.concatenate([q_cols, k_col, v_col], axis=1)
        wqkv_c = np.ascontiguousarray(
            wqkv_c.reshape(KH, 128, (QH + 2) * 128).transpose(1, 0, 2)).astype(bf16)
        wgu_c = np.concatenate(
            [w_gu_f[:, c * IPC:(c + 1) * IPC],
             w_gu_f[:, I + c * IPC: I + (c + 1) * IPC]], axis=1)
        wgu_c = np.ascontiguousarray(
            wgu_c.reshape(KH, 128, 2 * MB_GU, 128).transpose(1, 2, 0, 3)).astype(bf16)
        wdn_c = np.ascontiguousarray(
            w_dn_f[c * IPC:(c + 1) * IPC, :].reshape(KI, 128, KH, 128)
            .transpose(1, 2, 0, 3)).astype(bf16)
        hT_slice_c = np.ascontiguousarray(
            hT[:, c * TPC:(c + 1) * TPC].reshape(KH, 128, TPC)
            .transpose(1, 0, 2)).astype(ml_dtypes.bfloat16)
        in_maps.append({
            "hT": hT_bf,
            "hT_slice": hT_slice_c,
            "wqkv": np.ascontiguousarray(wqkv_c),
            "wo": w_o_f,
            "wgu": np.ascontiguousarray(wgu_c),
            "wdn": np.ascontiguousarray(wdn_c),
            "cosT": cosT_np,
            "sinT": sinT_np,
            "masks": masks_np,
        })
    return in_maps


def kernel(**inputs):
    in_maps = _host_prep(**inputs)
    nc = _get_nc()
    res = run_bass_kernel_spmd(nc, in_maps, core_ids=list(range(NC)))
    results = res.results

    outT = np.empty((H, S), np.float32)
    for c in range(NC):
        od = np.asarray(results[c]["out_down"]).astype(np.float32)  # [512, S]
        for r in range(8):
            outT[512 * r + 64 * c: 512 * r + 64 * (c + 1)] = od[64 * r:64 * (r + 1)]
    resT = np.concatenate(
        [np.asarray(results[c]["res_out"]).astype(np.float32) for c in range(NC)],
        axis=1)                                                      # [H, S]
    out = np.ascontiguousarray(outT.T).reshape(1, S, H).astype(np.float32)
    residual = np.ascontiguousarray(resT.T).reshape(1, S, H).astype(np.float32)
    return out, residual
